# revision 32
# baseline (speedup 1.0000x reference)
"""Trainium2 Bass kernel for nn_Diff_prop_18425409699925 (GNN message passing).

Math (per batch element b, with x = local_feat[b] reshaped to [n=1024, c=256]):
  xn   = x / ||x||_row
  A    = (xn * diag(W_adj)) @ xn^T                (symmetric; einsum uses only
                                                   the diagonal of W_adj)
  G    = exp(5*A) with diagonal zeroed            (the reference's row-max
                                                   shift cancels exactly in the
                                                   row-normalized mean)
  M    = (G @ x) / rowsum(G)
  diff = (x - M) @ W_aff^T + b_aff
  y    = LeakyReLU(LayerNorm(diff) * gamma + beta, 0.01)

Sharding: data-parallel over batch B=8 -> one batch element per NeuronCore,
weights replicated, no collectives. G (symmetric) is used directly as the
lhsT of the G @ x matmul, avoiding a [1024,1024] transpose.

All matmuls and big elementwise ops run in bf16 (operands; PSUM accumulation
is fp32).  Every ScalarE activation in the kernel (Exp, Ln, Square, Copy,
Prelu) lives in the single `natural_log_exp_and_others` table set, so the
~2.7us ACT table load is paid exactly once; sqrt/rsqrt/reciprocal are
computed as exp(-k*ln(s)).

The row-sum of diff (needed for the LayerNorm mean) is obtained for free by
appending the column-sums of W as an extra matmul column; likewise a ones
column on x makes the G @ x matmul emit rowsum(G).  LayerNorm is invariant
to positive per-row scaling, so the 1/rowsum(G) normalization of M is
dropped entirely (D'' = rowsum*x - G@x = rowsum*(x - M)), and LeakyReLU's
positive homogeneity moves the 1/std scaling after the activation.  Row
normalization is fused into the transposes as x^T @ diag(1/||x||).

global_feat and pos are unused by the reference; accepted and ignored.
"""

import os
import sys

import numpy as np

for _p in ("/opt/trn_rl_repo",):
    if os.path.isdir(_p) and _p not in sys.path:
        sys.path.insert(0, _p)

import ml_dtypes
import concourse.bacc as bacc
import concourse.bass as bass
import concourse.tile as tile
from concourse import mybir
from concourse.bass_utils import run_bass_kernel_spmd

B, T, NN, C = 8, 16, 64, 256
N = T * NN            # 1024 nodes per batch element
P = 128               # partitions
NT = N // P           # 8 n-tiles
CT = C // P           # 2 c-tiles
CX = C + 1            # D@W output incl. the rowsum column
XD = C + 2            # Xb row pitch: ones col + pad so 2-byte rows stay
                      # 4-byte aligned (514B pitch would break DVE 2x/4x)
F32 = mybir.dt.float32
BF16 = mybir.dt.bfloat16
TS = bass.ts
BF = ml_dtypes.bfloat16

LN_EPS = 1e-5
LEAK = 0.01
DIAG_NEG = -200.0     # added to diagonal of A pre-exp -> exp underflows to 0


def _steered_act_tables(orig_fn):
    """Steer the ACT table-set chooser: Exp and Ln both live in
    `natural_log_exp_and_others`, but the greedy chooser maps each function
    to the FIRST set containing it (natural_log for Ln, exp_and_others for
    Exp), thrashing a ~2.7us table load on every Ln<->Exp alternation.
    Emptying every other set makes the chooser settle on the one set that
    contains all activations this kernel uses -> exactly one load.  Set
    indices (and hence the emitted act_func_set_id) are unchanged."""
    def wrapped(arch):
        t = orig_fn(arch)
        AF = mybir.ActivationFunctionType
        if "natural_log_exp_and_others" in t:
            for nm in t:
                if nm != "natural_log_exp_and_others":
                    t[nm] = set()
        return t
    return wrapped


def _build_program(diag_one, ln_trivial):
    nc = bacc.Bacc("TRN2", target_bir_lowering=False, debug=False)
    _orig_gat = bacc.get_activation_tables
    bacc.get_activation_tables = _steered_act_tables(_orig_gat)
    try:
        return _build_program_inner(nc, diag_one, ln_trivial)
    finally:
        bacc.get_activation_tables = _orig_gat


def _build_program_inner(nc, diag_one, ln_trivial):

    xb_d = nc.declare_dram_parameter("xb", [P, NT, XD], BF16, isOutput=False)
    wt_d = nc.declare_dram_parameter("wt", [P, CT, CX], BF16, isOutput=False)
    identb_d = nc.declare_dram_parameter("identb", [P, P], BF16, isOutput=False)
    wdiag_d = nc.declare_dram_parameter("wdiag", [P, CT], F32, isOutput=False)
    rows_d = nc.declare_dram_parameter("rows", [1, 2 * C], F32, isOutput=False)
    bvec_d = nc.declare_dram_parameter("bvec", [1, CX], BF16, isOutput=False)
    y_d = nc.declare_dram_parameter("y", [P, NT, C], F32, isOutput=True)

    with tile.TileContext(nc) as tc:
        _emit(nc, tc, xb_d, wt_d, identb_d, wdiag_d, rows_d, bvec_d, y_d,
              diag_one, ln_trivial)
    nc.finalize()
    return nc


def _emit(nc, tc, xb_d, wt_d, identb_d, wdiag_d, rows_d, bvec_d, y_d,
          diag_one, ln_trivial):
    from contextlib import ExitStack

    mult = mybir.AluOpType.mult
    add = mybir.AluOpType.add
    subtract = mybir.AluOpType.subtract
    bypass = mybir.AluOpType.bypass
    amax = mybir.AluOpType.max
    lshr = mybir.AluOpType.logical_shift_right
    bnot = mybir.AluOpType.bitwise_not
    AF = mybir.ActivationFunctionType
    U32 = mybir.dt.uint32

    W0 = int(os.environ.get("KERNEL_WARMUP", "28"))
    WA = int(os.environ.get("KERNEL_WARMUP_A", "1"))
    WB = int(os.environ.get("KERNEL_WARMUP_B", "2"))
    WC = int(os.environ.get("KERNEL_WARMUP_C", "12"))
    use_prelu = bool(int(os.environ.get("KERNEL_PRELU", "1")))

    v = nc.vector
    s = nc.scalar
    te = nc.tensor
    sy = nc.sync
    gp = nc.gpsimd
    dma_eng = [sy, nc.scalar, gp]

    with ExitStack() as ctx:
        sb = ctx.enter_context(tc.tile_pool(name="sb", bufs=1))
        scr = ctx.enter_context(tc.tile_pool(name="scr", bufs=3))
        ps_a = ctx.enter_context(tc.tile_pool(name="ps_a", bufs=2, space="PSUM"))
        ps_c = ctx.enter_context(tc.tile_pool(name="ps_c", bufs=3, space="PSUM"))
        ps_t = ctx.enter_context(tc.tile_pool(name="ps_t", bufs=1, space="PSUM"))

        # ---------------- persistent SBUF tiles ----------------
        # Xb carries a trailing ones column so the G @ x matmul also yields
        # rowsum(G) (the L1 normalizer) in column 256 -- no reduction pass.
        Xb = sb.tile([P, NT, XD], BF16, tag="Xb", name="Xb")
        xnT = sb.tile([P, CT, N], BF16, tag="xnT", name="xnT")
        if diag_one:
            xnTs = xnT
        else:
            xnTs = sb.tile([P, CT, N], BF16, tag="xnTs", name="xnTs")
        G = [sb.tile([P, N], BF16, tag=f"G{i}", name=f"G{i}") for i in range(NT)]
        DTt = sb.tile([P, CT, N], BF16, tag="DTt", name="DTt")
        Y = sb.tile([P, NT, C], F32, tag="Y", name="Y")
        WT = sb.tile([P, CT, CX], BF16, tag="WT", name="WT")
        identb = sb.tile([P, P], BF16, tag="identb", name="identb")
        negeyeb = sb.tile([P, P], BF16, tag="negeyeb", name="negeyeb")
        warm_src = sb.tile([P, C], BF16, tag="warm_src", name="warm_src")

        # batched per-row stats, one column per n-tile
        def stat(nm):
            return sb.tile([P, NT], F32, tag=nm, name=nm)
        SS, LNS, RNO = stat("SS"), stat("LNS"), stat("RNO")
        SQ, MUn, MUSQ = stat("SQ"), stat("MUn"), stat("MUSQ")
        SQA, VAR, LV, RSTD, NB = (stat("SQA"), stat("VAR"), stat("LV"),
                                  stat("RSTD"), stat("NB"))

        if not diag_one:
            wdiag = sb.tile([P, CT], F32, tag="wdiag", name="wdiag")
        if not ln_trivial:
            g_bc = sb.tile([P, C], F32, tag="g_bc", name="g_bc")
            be_bc = sb.tile([P, C], F32, tag="be_bc", name="be_bc")
            rows = sb.tile([1, 2 * C], F32, tag="rows", name="rows")
            bvec = sb.tile([1, CX], BF16, tag="bvec", name="bvec")
            ones1f = sb.tile([1, P], F32, tag="ones1f", name="ones1f")
            ones1b = sb.tile([1, P], BF16, tag="ones1b", name="ones1b")

        # ---------------- loads (spread across DGE queues) ----------------
        # xb/wt/y are partition-major in DRAM (host handles the reshuffle),
        # so every partition's data is one contiguous descriptor.
        sy.dma_start(identb[:], identb_d[:])
        gp.dma_start(WT[:], wt_d[:])
        if not diag_one:
            gp.dma_start(wdiag[:], wdiag_d[:])
        if not ln_trivial:
            gp.dma_start(rows[:], rows_d[:])
            gp.dma_start(bvec[:], bvec_d[:])
        for qq in range(4):
            lo, hi = qq * 2, qq * 2 + 2
            (sy if qq % 2 == 0 else nc.scalar).dma_start(
                Xb[:, lo:hi, :], xb_d[:, lo:hi, :])

        v.memset(warm_src[:], 0.0)
        v.tensor_scalar_mul(negeyeb[:], identb[:], DIAG_NEG)

        if not ln_trivial:
            v.memset(ones1f[:], 1.0)
            v.memset(ones1b[:], 1.0)
            pg = ps_a.tile([P, N], F32, tag="pa", name="pg")
            nc.tensor.matmul(pg[:, 0:2 * C], ones1f[:], rows[:],
                             start=True, stop=True)
            v.tensor_copy(g_bc[:], pg[:, 0:C])
            v.tensor_copy(be_bc[:], pg[:, C:2 * C])

        # PE warm-up: keep the HAM clock gate open while the PE would
        # otherwise idle (input-DMA ramp, exp drain), so matmuls run at
        # 2.4 GHz instead of the cold 1.2 GHz.
        pw = None

        def warm(n):
            nonlocal pw
            if pw is None:
                pw = ps_a.tile([P, N], F32, tag="pa", name="pw")
            for _ in range(n):
                te.matmul(pw[:, 0:C], warm_src[:, 0:P], warm_src[:],
                          start=True, stop=True)

        warm(W0)

        # ---------------- phase A: row-normalize, build xn^T ----------------
        # quarter-batches track the 4 input-DMA chunks; rsqrt =
        # exp(-0.5*ln) on ScalarE, batched per quarter
        pa0 = None
        for hh in range(4):
            lo, hi = hh * 2, hh * 2 + 2
            for i in range(lo, hi):
                if hh == 3:
                    # last quarter is on the critical path into phase B:
                    # Square on ScalarE feeds Ln directly (no DVE->ACT hop)
                    sqf = scr.tile([P, C], F32, tag="sqf", name="sqf")
                    s.activation(sqf[:], Xb[:, i, 0:C], AF.Square,
                                 accum_out=SS[:, i:i + 1])
                else:
                    sqs = scr.tile([P, C], BF16, tag="sqs", name="sqs")
                    v.scalar_tensor_tensor(
                        out=sqs[:], in0=Xb[:, i, 0:C], scalar=1.0,
                        in1=Xb[:, i, 0:C],
                        op0=bypass, op1=mult, accum_out=SS[:, i:i + 1])
            s.activation(LNS[:, lo:hi], SS[:, lo:hi], AF.Ln)
            s.activation(RNO[:, lo:hi], LNS[:, lo:hi], AF.Exp, scale=-0.5)
            for i in range(lo, hi):
                xn = scr.tile([P, C], BF16, tag="xn", name=f"xn{i}",
                              bufs=3)
                v.tensor_scalar_mul(xn[:], Xb[:, i, 0:C], RNO[:, i:i + 1])
                pt = ps_t.tile([P, CT, P], BF16, tag="pt", name="pt")
                for k in range(CT):
                    te.transpose(pt[:, k, :], xn[:, TS(k, P)], identb[:])
                warm(WA)
                dst = xnT[:, :, TS(i, P)]
                if i % 2 == 0:
                    s.activation(dst, pt[:], AF.Copy)
                else:
                    v.tensor_copy(dst, pt[:])
                if not diag_one:
                    for k in range(CT):
                        v.tensor_scalar_mul(
                            xnTs[:, k, TS(i, P)], pt[:, k, :],
                            wdiag[:, k:k + 1])
            if hh == 1:
                # head-start: row 0 x columns 0-511 of the affinity matmul
                # needs only the first four xn^T tiles.  Allocation order of
                # the pa ring is unchanged (pa0 is still the alloc after pw).
                pa0 = ps_a.tile([P, N], F32, tag="pa", name="pa0")
                for k in range(CT):
                    te.matmul(pa0[:, TS(0, 512)], xnTs[:, k, TS(0, P)],
                              xnT[:, k, TS(0, 512)],
                              start=(k == 0), stop=False)

        # ---------------- phase B: A = xnTs^T @ xnT, G = exp(5A) ----------------
        # The -200*I diagonal knockout is an extra accumulating matmul, so
        # exp depends only on the PE and ScalarE streams exp back-to-back.
        for i in range(NT):
            jd = i // 4
            if i == 0 and pa0 is not None:
                pa = pa0
                jrange = [1]
            else:
                pa = ps_a.tile([P, N], F32, tag="pa", name=f"pa{i}")
                jrange = [0, 1]
            for j in jrange:
                for k in range(CT):
                    te.matmul(
                        pa[:, TS(j, 512)],
                        xnTs[:, k, TS(i, P)],
                        xnT[:, k, TS(j, 512)],
                        start=(k == 0),
                        stop=(k == CT - 1) and (j != jd))
            te.matmul(pa[:, TS(i, P)], negeyeb[:], identb[:],
                      start=False, stop=True)
            warm(WB)
            s.activation(G[i][:], pa[:], AF.Exp, scale=5.0)
        warm(WC)

        # ---------------- phase C: M, diff = (x-M)W^T, LN, LeakyReLU ----------------
        # 3-stage software pipeline with a 2-block skew: S1 = G@x + 1/rowsum
        # + D', S2 (two blocks later) = transpose + D@W + Square + Prelu(pd-mu),
        # S3 (per 4-tile group) = batched rstd + scale + store.  LeakyReLU is
        # positively homogeneous, so the 1/std scaling happens after the
        # activation on DVE and pd's PSUM lifetime ends inside S2.  Engine
        # queues are ordered by data readiness (transpose first on PE, copy
        # first on DVE) so nothing head-blocks.
        y_g = y_d
        Dbs = [None] * NT
        qs = [None] * NT
        qpool = ctx.enter_context(tc.tile_pool(name="qpool", bufs=5))

        def finish_group(i0, n):
            s.activation(LV[:, i0:i0 + n], VAR[:, i0:i0 + n], AF.Ln)
            s.activation(RSTD[:, i0:i0 + n], LV[:, i0:i0 + n], AF.Exp,
                         scale=-0.5)
            for i in range(i0, i0 + n):
                if ln_trivial:
                    v.tensor_scalar_mul(Y[:, i, :], qs[i], RSTD[:, i:i + 1])
                else:
                    # q = diff - mu; t = q*rstd, then gamma/beta + leaky
                    u = scr.tile([P, C], F32, tag="u", name="u")
                    v.tensor_scalar_mul(u[:], qs[i], RSTD[:, i:i + 1])
                    uu = scr.tile([P, C], F32, tag="uu", name="uu")
                    v.tensor_mul(uu[:], u[:], g_bc[:])
                    w_ = scr.tile([P, C], F32, tag="w_", name="w_")
                    v.tensor_add(w_[:], uu[:], be_bc[:])
                    v.scalar_tensor_tensor(
                        out=Y[:, i, :], in0=w_[:], scalar=LEAK, in1=w_[:],
                        op0=mult, op1=amax)
                sy.dma_start(y_g[:, i, :], Y[:, i, :])

        for ii in range(NT + 2):
            j = ii - 2  # S2 tile index
            if 0 <= j:
                # PE: transpose first -- Db_j has been ready for a block
                Db = Dbs[j]
                ptd = ps_t.tile([P, CT, P], BF16, tag="pt", name=f"ptd{j}")
                for k in range(CT):
                    te.transpose(ptd[:, k, :], Db[:, TS(k, P)], identb[:])
            if ii < NT:
                i = ii
                py = ps_c.tile([P, CX], F32, tag="pc", name=f"py{i}")
                for k in range(NT):
                    te.matmul(py[:], G[k][:, TS(i, P)], Xb[:, k, 0:CX],
                              start=(k == 0), stop=(k == NT - 1))
            if 0 <= j:
                v.tensor_copy(DTt[:, :, TS(j, P)], ptd[:])
                pd = ps_c.tile([P, CX], F32, tag="pc", name=f"pd{j}")
                have_b = not ln_trivial
                for k in range(CT):
                    te.matmul(pd[:], DTt[:, k, TS(j, P)], WT[:, k, :],
                              start=(k == 0),
                              stop=(k == CT - 1) and not have_b)
                if have_b:
                    te.matmul(pd[:], ones1b[:], bvec[:], start=False,
                              stop=True)
                sqo = scr.tile([P, C], F32, tag="sqo", name="sqo")
                s.activation(sqo[:], pd[:, 0:C], AF.Square,
                             accum_out=SQ[:, j:j + 1])
            if 0 <= j:
                v.tensor_scalar_mul(MUn[:, j:j + 1], pd[:, C:CX], -1.0 / C)
                v.tensor_mul(MUSQ[:, j:j + 1], MUn[:, j:j + 1],
                             MUn[:, j:j + 1])
                v.tensor_scalar(
                    out=SQA[:, j:j + 1], in0=SQ[:, j:j + 1],
                    scalar1=1.0 / C, scalar2=LN_EPS, op0=mult, op1=add)
                v.scalar_tensor_tensor(
                    out=VAR[:, j:j + 1], in0=SQA[:, j:j + 1], scalar=1.0,
                    in1=MUSQ[:, j:j + 1], op0=bypass, op1=subtract)
                q = qpool.tile([P, C], F32, tag="q", name=f"q{j}")
                qs[j] = q
                if ln_trivial and use_prelu:
                    s.activation(q[:], pd[:, 0:C], AF.Prelu,
                                 bias=MUn[:, j:j + 1], alpha=LEAK)
                elif ln_trivial:
                    # CoreSim fallback: Prelu isn't implemented there
                    tt = scr.tile([P, C], F32, tag="tt", name="tt")
                    s.activation(tt[:], pd[:, 0:C], AF.Identity,
                                 bias=MUn[:, j:j + 1])
                    v.scalar_tensor_tensor(
                        out=q[:], in0=tt[:], scalar=LEAK, in1=tt[:],
                        op0=mult, op1=amax)
                else:
                    # generic path: keep (diff - mu) unactivated; gamma/beta
                    # and the leaky relu are applied in finish_group
                    s.activation(q[:], pd[:, 0:C], AF.Identity,
                                 bias=MUn[:, j:j + 1])
            if ii < NT:
                i = ii
                # LayerNorm is row-scale-invariant, so skip the 1/rowsum(G)
                # normalization: D'' = x*rowsum - G@x = rowsum*(x - M), and
                # rowsum comes straight out of py column 256 (PSUM scalar).
                Db = scr.tile([P, C], BF16, tag="db", name=f"db{i}")
                v.scalar_tensor_tensor(
                    out=Db[:], in0=Xb[:, i, 0:C], scalar=py[:, C:CX],
                    in1=py[:, 0:C], op0=mult, op1=subtract)
                Dbs[i] = Db
            if j == 3:
                finish_group(0, 4)
            elif j == 5:
                finish_group(4, 2)
            elif j in (6, 7):
                finish_group(j, 1)


_PROGRAM_CACHE = {}
last_results = None


def _get_program(diag_one=True, ln_trivial=True):
    key = (diag_one, ln_trivial,
           os.environ.get("KERNEL_WARMUP", "48"),
           os.environ.get("KERNEL_WARMUP_A", "2"),
           os.environ.get("KERNEL_WARMUP_B", "2"),
           os.environ.get("KERNEL_WARMUP_C", "12"),
           os.environ.get("KERNEL_PRELU", "1"))
    if key not in _PROGRAM_CACHE:
        _PROGRAM_CACHE[key] = _build_program(diag_one, ln_trivial)
    return _PROGRAM_CACHE[key]


def _prep_inputs(local_feat, W_adj, W_aff, b_aff, ln_gamma, ln_beta):
    x = np.asarray(local_feat, np.float32).reshape(B, N, C)
    # partition-major layout with a trailing ones column: xb_r[b, p, i, :] =
    # [x[b, i*128+p, :], 1.0] -- every SBUF partition's data is one
    # contiguous DMA descriptor, and the ones column makes the G @ x matmul
    # also produce rowsum(G).
    xb = np.ones((B, P, NT, XD), dtype=BF)
    xb[:, :, :, 0:C] = x.reshape(B, NT, P, C).transpose(0, 2, 1, 3).astype(BF)
    Wf = np.asarray(W_aff, np.float32)
    wpos = np.ascontiguousarray(Wf.T).astype(BF)         # [cin, cout]
    w1 = wpos.astype(np.float32).sum(axis=1).astype(BF)  # rowsum column
    wt = np.concatenate([wpos, w1[:, None]], axis=1)     # [C, 257] bf16
    wt_r = np.ascontiguousarray(
        wt.reshape(CT, P, CX).transpose(1, 0, 2))        # [P, CT, 257]
    identb = np.eye(P, dtype=np.float32).astype(BF)
    diag = np.ascontiguousarray(np.diagonal(np.asarray(W_adj, np.float32)))
    wd = np.ascontiguousarray(diag.reshape(CT, P).T).astype(np.float32)
    b = np.asarray(b_aff, np.float32).ravel()
    g = np.asarray(ln_gamma, np.float32).ravel()
    be = np.asarray(ln_beta, np.float32).ravel()
    rows = np.concatenate([g, be]).reshape(1, 2 * C).astype(np.float32)
    bvec = np.concatenate([b, [b.sum()]]).reshape(1, CX).astype(BF)
    diag_one = bool(np.all(diag == 1.0))
    ln_trivial = bool(np.all(g == 1.0) and np.all(be == 0.0)
                      and np.all(b == 0.0))
    in_maps = [
        {"xb": np.ascontiguousarray(xb[bb]), "wt": wt_r, "identb": identb,
         "wdiag": wd, "rows": rows, "bvec": bvec}
        for bb in range(B)
    ]
    return in_maps, diag_one, ln_trivial


def kernel(local_feat, global_feat, pos, W_adj, W_aff, b_aff, ln_gamma,
           ln_beta, **_unused):
    global last_results
    in_maps, diag_one, ln_trivial = _prep_inputs(
        local_feat, W_adj, W_aff, b_aff, ln_gamma, ln_beta)
    nc = _get_program(diag_one, ln_trivial)
    trace = bool(int(os.environ.get("KERNEL_TRACE", "0")))
    res = run_bass_kernel_spmd(nc, in_maps, list(range(B)), trace=trace)
    last_results = res
    out = np.stack([np.asarray(res.results[bb]["y"]) for bb in range(B)],
                   axis=0)                                # [B, P, NT, C]
    out = out.transpose(0, 2, 1, 3).reshape(B, N, C)      # n = i*128 + p
    return out.reshape(B, T, NN, C).astype(np.float32)
